# revision 2
# baseline (speedup 1.0000x reference)
"""Trainium2 Bass kernel for CumulantSOAP_CV — PE-only DoubleRow design.

reference math:
    m    = mean(X, axis=0)                       # (576,)
    mom1 = mean(X - m, axis=0)  (~0)             # (576,)
    mom2 = mean((X - m)^2, axis=0)               # (576,)
    cum  = interleave(m, mom1, mom2)             # (1, 1728)
    out  = (cum - mu) @ W                        # (1, 4)

Only the raw column moments S1 = sum(x) and S2 = sum(x^2) need the full
data; everything after is a tiny host-side fixup.  The tolerance (2e-2)
is ~100x looser than fp8e4 (e4m3) costs end-to-end (~2e-4), so the
kernel ships X as 1-byte e4m3 and does ALL the reduction work on the PE
with fp8 DoubleRow matmuls (measured 1.95x the regular fp8 rate:
391 ns per 256-row superblock vs 762).

Layout: 256-row superblocks, rows on partitions as (k=2, p=128), row
slots of 608 bytes (4x(128 data + 1.0 + 3 pad) + (64 data + 1.0) + 12
pad; 608 %% 16 == 0 satisfies the dual-fp8 LDWEIGHTS step restriction).
For each superblock and column group g (widths 128,128,128,128,64):
    acc_g += sum_k X[k][:, g]^T @ [X[k][:, g] | 1]
via one DoubleRow matmul (lhsT=[128,2,w], rhs=[128,2,w+1], 256-deep
contraction).  diag(acc_g) = sum(x^2), last column = sum(x).  Matmuls
are group-major per DMA tile (consecutive MMs on one PSUM bank).

The DMA stream is a single in-order sync-ring queue of 12 per-tile
dram tensors (one linear HBM read each): 4-superblock head tiles so
the PE starts after ~600 KB, then 8..12-superblock bodies (9.7-14.6 KB
lines).  The PE consumes ~5-10% slower per tile than the DMA delivers,
so once started it never idles mid-stream -- important because PE-idle
gaps >~1.5 us re-trigger HAM half-clock windows (measured +3.4 us
each).  A trailing dummy re-read keeps the queue deep while the last
real tensors drain (the final ~2 MB otherwise trickles at <100 GB/s
once every core's queue goes shallow; measured ~5 us), and its own
tail overlaps the compute/output tail.  Outputs ride the scalar ring
so they never queue behind it.  Every tile has a dedicated SBUF buffer
(~134 KB/partition total) so the DMA never waits on compute.  The 8
per-core Gram outputs are combined on host in f64.
"""

import sys
import types

import numpy as np

N_CORES = 8
N_ROWS = 200000
P = 576
PROJ_DIMS = 4
ROWS_PER_CORE = N_ROWS // N_CORES   # 25000
PART = 128

ROWB = 608                          # padded row bytes (16B multiple)
SB = 98                             # 256-row superblocks (25088 rows, 88 pad)
G_TILES = [8, 8, 8, 8, 8, 8, 8, 8, 8, 8, 8, 10]   # superblocks per DMA tile
assert sum(G_TILES) == SB
GW = [128, 128, 128, 128, 64]       # column-group widths (sum = 576)
GOFF_IN = [0, 132, 264, 396, 528]   # group offsets in the padded row
GOFF_OUT = [0, 129, 258, 387]       # group offsets in out0 (g0..g3)
OUT0W = 516                         # 4*129


def _build():
    import concourse.bacc as bacc
    import concourse.mybir as mybir
    import concourse.tile as tile

    nc = bacc.Bacc(None, target_bir_lowering=False)
    f32 = mybir.dt.float32
    f8 = mybir.dt.float8e4
    xs = [
        nc.dram_tensor(f"x{t}", [PART, nb * 2 * ROWB], f8, kind="ExternalInput")
        for t, nb in enumerate(G_TILES)
    ]
    out0 = nc.dram_tensor("out0", [PART, OUT0W], f32, kind="ExternalOutput")
    out1 = nc.dram_tensor("out1", [GW[4], GW[4] + 1], f32, kind="ExternalOutput")

    with tile.TileContext(nc) as tc:
        with (
            tc.tile_pool(name="xp", bufs=1) as xp,
            tc.tile_pool(name="op", bufs=1) as op,
            tc.tile_pool(name="ps", bufs=1, space="PSUM") as ps,
        ):
            acc = [
                ps.tile([GW[g], GW[g] + 1], f32, name=f"acc{g}", tag=f"acc{g}")
                for g in range(5)
            ]
            sb0 = 0
            for t, nb in enumerate(G_TILES):
                xt = xp.tile([PART, 2 * nb, ROWB], f8, name=f"xt{t}", tag=f"xt{t}")
                nc.sync.dma_start(out=xt[:], in_=xs[t][:])
                for g in range(5):
                    o, w = GOFF_IN[g], GW[g]
                    for b in range(nb):
                        sb = sb0 + b
                        nc.tensor.matmul(
                            acc[g][:],
                            xt[:, 2 * b:2 * b + 2, o:o + w],
                            xt[:, 2 * b:2 * b + 2, o:o + w + 1],
                            start=sb == 0,
                            stop=sb == SB - 1,
                            perf_mode=mybir.MatmulPerfMode.DoubleRow,
                        )
                sb0 += nb

            # trailing dummy re-read keeps the sync queue deep while the
            # last real tensors drain: without it the final ~2 MB trickles
            # at <100 GB/s once every core's queue goes shallow (measured
            # ~5 us loss).  Its own tail overlaps the compute/output tail.
            dummy = xp.tile([PART, 2 * G_TILES[-1], ROWB], f8, name="dummy", tag="dummy")
            nc.sync.dma_start(out=dummy[:], in_=xs[len(G_TILES) - 1][:])

            # g0..g3 copies + their DMA overlap the tail of g4's matmuls
            ot0 = op.tile([PART, OUT0W], f32)
            ot1 = op.tile([GW[4], GW[4] + 1], f32)
            for g, eng in zip(range(4), ("vector", "scalar") * 2):
                dst = ot0[0:GW[g], GOFF_OUT[g]:GOFF_OUT[g] + GW[g] + 1]
                copy = nc.vector.tensor_copy if eng == "vector" else nc.scalar.copy
                copy(dst, acc[g][:])
            nc.scalar.dma_start(out=out0[:], in_=ot0[:])
            nc.vector.tensor_copy(ot1[:], acc[4][:])
            nc.scalar.dma_start(out=out1[:], in_=ot1[:])
    nc.compile()
    return nc


def _pack_cores(X):
    """(200000, 576) f32 -> per-dram-tensor contiguous e4m3 shards."""
    import ml_dtypes

    f8 = ml_dtypes.float8_e4m3fn
    Xq = X.astype(f8).reshape(N_CORES, ROWS_PER_CORE, P)
    A = np.zeros((N_CORES, SB, 2, PART, ROWB), dtype=f8)
    rows = SB * 2 * PART                         # 25088
    full = ROWS_PER_CORE // PART                 # 195 full 128-row blocks
    rem = ROWS_PER_CORE - full * PART            # 40 rows in the last block
    B = A.reshape(N_CORES, SB * 2, PART, ROWB)   # 196 blocks of 128 rows
    c0 = 0
    for g in range(5):
        o, w = GOFF_IN[g], GW[g]
        B[:, :full, :, o:o + w] = Xq[:, :full * PART].reshape(
            N_CORES, full, PART, P
        )[..., c0:c0 + w]
        B[:, full, :rem, o:o + w] = Xq[:, full * PART:, c0:c0 + w]
        B[:, :, :, o + w] = f8(1.0)
        c0 += w
    shards = {}
    b0 = 0
    for t, nb in enumerate(G_TILES):
        shards[f"x{t}"] = np.ascontiguousarray(
            A[:, b0:b0 + nb].transpose(0, 3, 1, 2, 4)
        ).reshape(N_CORES, PART, nb * 2 * ROWB)
        b0 += nb
    return shards


def _install_ntff_hook():
    """This image's antenv lacks axon_hooks, which bass_utils imports when
    tracing is requested (trace=True or BASS_TRACE=1).  Recreate the module
    from the injected libaxon_pjrt.so so tracing works instead of crashing.
    Harmless when tracing is off."""
    try:
        import antenv.axon_hooks  # noqa: F401
        return
    except ImportError:
        pass
    try:
        import antenv
        import trn_agent_boot.trn_boot as tb

        hook = tb._ntff_profile_via_ctypes("/opt/axon/libaxon_pjrt.so")
        mod = types.ModuleType("antenv.axon_hooks")
        mod._hook = hook
        mod.get_axon_ntff_profile_hook = lambda: mod._hook
        mod.set_axon_ntff_profile_hook = lambda h: None
        sys.modules["antenv.axon_hooks"] = mod
        antenv.axon_hooks = mod
    except Exception:
        pass


def _run_device(X, trace=False, **run_kwargs):
    from concourse.bass_utils import run_bass_kernel_spmd

    _install_ntff_hook()
    nc = _build()
    shards = _pack_cores(X)
    in_maps = [
        {k: v[c] for k, v in shards.items()} for c in range(N_CORES)
    ]
    res = run_bass_kernel_spmd(
        nc, in_maps, list(range(N_CORES)), trace=trace, **run_kwargs
    )
    p0 = np.stack([np.asarray(r["out0"], dtype=np.float32) for r in res.results])
    p1 = np.stack([np.asarray(r["out1"], dtype=np.float32) for r in res.results])
    return (p0, p1), res


def _finish(partials, mu, W):
    S1 = np.zeros(P, dtype=np.float64)
    S2 = np.zeros(P, dtype=np.float64)
    p0, p1 = partials
    g0 = p0.astype(np.float64).sum(axis=0)       # (128, OUT0W)
    c0 = 0
    for g in range(4):
        o, w = GOFF_OUT[g], GW[g]
        blk = g0[:w, o:o + w + 1]
        idx = np.arange(w)
        S2[c0:c0 + w] += blk[idx, idx]
        S1[c0:c0 + w] += blk[:, w]
        c0 += w
    w = GW[4]
    blk = p1.astype(np.float64).sum(axis=0)      # (64, 65)
    idx = np.arange(w)
    S2[c0:c0 + w] += blk[idx, idx]
    S1[c0:c0 + w] += blk[:, w]
    n = float(N_ROWS)
    m = S1 / n
    mom2 = S2 / n - m * m
    cum = np.stack([m, np.zeros_like(m), mom2], axis=1).reshape(-1)  # (1728,)
    proj = (cum - mu.astype(np.float64)) @ W.astype(np.float64)
    return proj.astype(np.float32).reshape(1, PROJ_DIMS)


def kernel(X, mu, W):
    X = np.asarray(X, dtype=np.float32)
    mu = np.asarray(mu, dtype=np.float32)
    W = np.asarray(W, dtype=np.float32)
    partials, _ = _run_device(X)
    return _finish(partials, mu, W)


# revision 3
# speedup vs baseline: 1.0641x; 1.0641x over previous
"""Trainium2 Bass kernel for CumulantSOAP_CV — PE-only DoubleRow design.

reference math:
    m    = mean(X, axis=0)                       # (576,)
    mom1 = mean(X - m, axis=0)  (~0)             # (576,)
    mom2 = mean((X - m)^2, axis=0)               # (576,)
    cum  = interleave(m, mom1, mom2)             # (1, 1728)
    out  = (cum - mu) @ W                        # (1, 4)

Only the raw column moments S1 = sum(x) and S2 = sum(x^2) need the full
data; everything after is a tiny host-side fixup.  The tolerance (2e-2)
is ~100x looser than fp8e4 (e4m3) costs end-to-end (~2e-4), so the
kernel ships X as 1-byte e4m3 and does ALL the reduction work on the PE
with fp8 DoubleRow matmuls (measured 1.95x the regular fp8 rate:
391 ns per 256-row superblock vs 762).

Layout: 256-row superblocks, rows on partitions as (k=2, p=128), row
slots of 608 bytes (4x(128 data + 1.0 + 3 pad) + (64 data + 1.0) + 12
pad; 608 %% 16 == 0 satisfies the dual-fp8 LDWEIGHTS step restriction).
For each superblock and column group g (widths 128,128,128,128,64):
    acc_g += sum_k X[k][:, g]^T @ [X[k][:, g] | 1]
via one DoubleRow matmul (lhsT=[128,2,w], rhs=[128,2,w+1], 256-deep
contraction).  diag(acc_g) = sum(x^2), last column = sum(x).  Matmuls
are group-major per DMA tile (consecutive MMs on one PSUM bank).

The DMA stream is a single in-order sync-ring queue of 12 per-tile
dram tensors (one linear HBM read each), uniform 8-superblock tiles
(9.7 KB lines).  The PE consumes slightly slower per tile than the
DMA delivers, so once started it rarely idles mid-stream -- important because PE-idle
gaps >~1.5 us re-trigger HAM half-clock windows (measured +3.4 us
each).  A trailing dummy re-read keeps the queue deep while the last
real tensors drain (the final ~2 MB otherwise trickles at <100 GB/s
once every core's queue goes shallow; measured ~5 us), and its own
tail overlaps the compute/output tail.  Outputs ride the scalar ring
so they never queue behind it.  Every tile has a dedicated SBUF buffer
(~134 KB/partition total) so the DMA never waits on compute.  The 8
per-core Gram outputs are combined on host in f64.
"""

import sys
import types

import numpy as np

N_CORES = 8
N_ROWS = 200000
P = 576
PROJ_DIMS = 4
ROWS_PER_CORE = N_ROWS // N_CORES   # 25000
PART = 128

ROWB = 608                          # padded row bytes (16B multiple)
SB = 98                             # 256-row superblocks (25088 rows, 88 pad)
G_TILES = [8, 8, 8, 8, 8, 8, 8, 8, 8, 8, 8, 10]   # superblocks per DMA tile
assert sum(G_TILES) == SB
GW = [128, 128, 128, 128, 64]       # column-group widths (sum = 576)
GOFF_IN = [0, 132, 264, 396, 528]   # group offsets in the padded row
GOFF_OUT = [0, 129, 258, 387]       # group offsets in out0 (g0..g3)
OUT0W = 516                         # 4*129


def _build():
    import concourse.bacc as bacc
    import concourse.mybir as mybir
    import concourse.tile as tile

    nc = bacc.Bacc(None, target_bir_lowering=False)
    f32 = mybir.dt.float32
    f8 = mybir.dt.float8e4
    xs = [
        nc.dram_tensor(f"x{t}", [PART, nb * 2 * ROWB], f8, kind="ExternalInput")
        for t, nb in enumerate(G_TILES)
    ]
    out0 = nc.dram_tensor("out0", [PART, OUT0W], f32, kind="ExternalOutput")
    out1 = nc.dram_tensor("out1", [GW[4], GW[4] + 1], f32, kind="ExternalOutput")

    with tile.TileContext(nc) as tc:
        with (
            tc.tile_pool(name="xp", bufs=1) as xp,
            tc.tile_pool(name="op", bufs=1) as op,
            tc.tile_pool(name="ps", bufs=1, space="PSUM") as ps,
        ):
            acc = [
                ps.tile([GW[g], GW[g] + 1], f32, name=f"acc{g}", tag=f"acc{g}")
                for g in range(5)
            ]
            sb0 = 0
            for t, nb in enumerate(G_TILES):
                xt = xp.tile([PART, 2 * nb, ROWB], f8, name=f"xt{t}", tag=f"xt{t}")
                nc.sync.dma_start(out=xt[:], in_=xs[t][:])
                for g in range(5):
                    o, w = GOFF_IN[g], GW[g]
                    for b in range(nb):
                        sb = sb0 + b
                        nc.tensor.matmul(
                            acc[g][:],
                            xt[:, 2 * b:2 * b + 2, o:o + w],
                            xt[:, 2 * b:2 * b + 2, o:o + w + 1],
                            start=sb == 0,
                            stop=sb == SB - 1,
                            perf_mode=mybir.MatmulPerfMode.DoubleRow,
                        )
                sb0 += nb

            # trailing dummy re-read keeps the sync queue deep while the
            # last real tensors drain: without it the final ~2 MB trickles
            # at <100 GB/s once every core's queue goes shallow (measured
            # ~5 us loss).  Its own tail overlaps the compute/output tail.
            dummy = xp.tile([PART, 2 * G_TILES[-1], ROWB], f8, name="dummy", tag="dummy")
            nc.sync.dma_start(out=dummy[:], in_=xs[len(G_TILES) - 1][:])

            # g0..g3 copies + their DMA overlap the tail of g4's matmuls
            ot0 = op.tile([PART, OUT0W], f32)
            ot1 = op.tile([GW[4], GW[4] + 1], f32)
            for g, eng in zip(range(4), ("vector", "scalar") * 2):
                dst = ot0[0:GW[g], GOFF_OUT[g]:GOFF_OUT[g] + GW[g] + 1]
                copy = nc.vector.tensor_copy if eng == "vector" else nc.scalar.copy
                copy(dst, acc[g][:])
            nc.scalar.dma_start(out=out0[:], in_=ot0[:])
            nc.vector.tensor_copy(ot1[:], acc[4][:])
            nc.scalar.dma_start(out=out1[:], in_=ot1[:])
    nc.compile()
    return nc


def _pack_cores(X):
    """(200000, 576) f32 -> per-dram-tensor contiguous e4m3 shards."""
    import ml_dtypes

    f8 = ml_dtypes.float8_e4m3fn
    Xq = X.astype(f8).reshape(N_CORES, ROWS_PER_CORE, P)
    A = np.zeros((N_CORES, SB, 2, PART, ROWB), dtype=f8)
    rows = SB * 2 * PART                         # 25088
    full = ROWS_PER_CORE // PART                 # 195 full 128-row blocks
    rem = ROWS_PER_CORE - full * PART            # 40 rows in the last block
    B = A.reshape(N_CORES, SB * 2, PART, ROWB)   # 196 blocks of 128 rows
    c0 = 0
    for g in range(5):
        o, w = GOFF_IN[g], GW[g]
        B[:, :full, :, o:o + w] = Xq[:, :full * PART].reshape(
            N_CORES, full, PART, P
        )[..., c0:c0 + w]
        B[:, full, :rem, o:o + w] = Xq[:, full * PART:, c0:c0 + w]
        B[:, :, :, o + w] = f8(1.0)
        c0 += w
    shards = {}
    b0 = 0
    for t, nb in enumerate(G_TILES):
        shards[f"x{t}"] = np.ascontiguousarray(
            A[:, b0:b0 + nb].transpose(0, 3, 1, 2, 4)
        ).reshape(N_CORES, PART, nb * 2 * ROWB)
        b0 += nb
    return shards


def _install_ntff_hook():
    """This image's antenv lacks axon_hooks, which bass_utils imports when
    tracing is requested (trace=True or BASS_TRACE=1).  Recreate the module
    from the injected libaxon_pjrt.so so tracing works instead of crashing.
    Harmless when tracing is off."""
    try:
        import antenv.axon_hooks  # noqa: F401
        return
    except ImportError:
        pass
    try:
        import antenv
        import trn_agent_boot.trn_boot as tb

        hook = tb._ntff_profile_via_ctypes("/opt/axon/libaxon_pjrt.so")
        mod = types.ModuleType("antenv.axon_hooks")
        mod._hook = hook
        mod.get_axon_ntff_profile_hook = lambda: mod._hook
        mod.set_axon_ntff_profile_hook = lambda h: None
        sys.modules["antenv.axon_hooks"] = mod
        antenv.axon_hooks = mod
    except Exception:
        pass


def _run_device(X, trace=False, **run_kwargs):
    from concourse.bass_utils import run_bass_kernel_spmd

    _install_ntff_hook()
    nc = _build()
    shards = _pack_cores(X)
    in_maps = [
        {k: v[c] for k, v in shards.items()} for c in range(N_CORES)
    ]
    res = run_bass_kernel_spmd(
        nc, in_maps, list(range(N_CORES)), trace=trace, **run_kwargs
    )
    p0 = np.stack([np.asarray(r["out0"], dtype=np.float32) for r in res.results])
    p1 = np.stack([np.asarray(r["out1"], dtype=np.float32) for r in res.results])
    return (p0, p1), res


def _finish(partials, mu, W):
    S1 = np.zeros(P, dtype=np.float64)
    S2 = np.zeros(P, dtype=np.float64)
    p0, p1 = partials
    g0 = p0.astype(np.float64).sum(axis=0)       # (128, OUT0W)
    c0 = 0
    for g in range(4):
        o, w = GOFF_OUT[g], GW[g]
        blk = g0[:w, o:o + w + 1]
        idx = np.arange(w)
        S2[c0:c0 + w] += blk[idx, idx]
        S1[c0:c0 + w] += blk[:, w]
        c0 += w
    w = GW[4]
    blk = p1.astype(np.float64).sum(axis=0)      # (64, 65)
    idx = np.arange(w)
    S2[c0:c0 + w] += blk[idx, idx]
    S1[c0:c0 + w] += blk[:, w]
    n = float(N_ROWS)
    m = S1 / n
    mom2 = S2 / n - m * m
    cum = np.stack([m, np.zeros_like(m), mom2], axis=1).reshape(-1)  # (1728,)
    proj = (cum - mu.astype(np.float64)) @ W.astype(np.float64)
    return proj.astype(np.float32).reshape(1, PROJ_DIMS)


def kernel(X, mu, W):
    X = np.asarray(X, dtype=np.float32)
    mu = np.asarray(mu, dtype=np.float32)
    W = np.asarray(W, dtype=np.float32)
    partials, _ = _run_device(X)
    return _finish(partials, mu, W)
